# revision 1
# baseline (speedup 1.0000x reference)
"""ArcFace loss on 8 TRN2 NeuronCores — class-parallel (vocab-sharded), fp8.

Math: loss = mean_b[ M0 + ln(Z'_b) - s*phi_b ] with
  Z_b  = sum_c exp(s*cos(b,c) - M0)          (device, sharded over classes)
  Z'_b = Z_b - exp(s*cosq(b,l_b) - M0) + exp(s*phi_b - M0)
cosq is the device's fp8 cosine for the label class (host replicates the
fp8 dot so the correction cancels the device term); phi uses the exact
f64 cosine. M0 is a fixed logsumexp shift: |cos| <= ~1.07 even with fp8
rounding, so s*cos - M0 <= 89 and exp() never overflows f32.

Device: x and W rows are unit-normalized on host, scaled by 32 and
quantized to fp8 e4m3 (max |elem| = 32 << 448), so the 512x512x100k
matmul runs in DoubleRow perf mode (2 fp8 weights per PE cell, two
128-row contraction chunks per instruction). The Exp runs on the scalar
engine in 2048-wide instructions reading PSUM, writing bf16 to SBUF
without accum_out; the vector engine folds each chunk in half with one
tensor_tensor_reduce (half the reduce cycles of a plain reduce). Each
core DMAs out just its [128, 4] partial-Z block; the cross-core sum
(4 KB), label correction, ln and mean run on the host in f64 — no
device collective at all.
"""

import math

import numpy as np

from concourse import bacc, mybir
from concourse.bass_utils import run_bass_kernel_spmd
from concourse.tile import TileContext

NCORES = 8
B = 512
D = 512
C = 100000
CS = 12544  # per-core classes, padded: 8 * 12544 = 100352 >= C
S = 120.0
MARGIN = 0.3
COS_M = math.cos(MARGIN)
SIN_M = math.sin(MARGIN)
TH = math.cos(math.pi - MARGIN)
MM = math.sin(math.pi - MARGIN) * MARGIN
M0 = 40.0  # logsumexp shift
QS = 32.0  # fp8 quantization scale for x and W (unit rows -> |elem*QS| <= 32)
SUPER = 2048  # class columns per superblock (one Exp instruction)
NBLK = 512  # class columns per matmul (one PSUM bank)
SBS = [SUPER] * 6 + [256]  # superblock widths; sum == CS
assert sum(SBS) == CS

F32 = mybir.dt.float32
BF16 = mybir.dt.bfloat16
F8 = mybir.dt.float8e4
FN = mybir.ActivationFunctionType
DR = mybir.MatmulPerfMode.DoubleRow
ADD = mybir.AluOpType.add
USE_TTR = True  # fused half-fold tensor_tensor_reduce vs plain reduce_sum

_GRAPH = None
LAST_RESULT = None  # BassKernelResults of the most recent run (for test harness)


def _build_nc(repeat=1):
    """Build the SPMD graph. repeat>1 unrolls the whole body N times into one
    NEFF (timing only: amortizes the per-execute dispatch overhead)."""
    nc = bacc.Bacc("TRN2", target_bir_lowering=False)

    # const AP for the Exp bias (only 0.0/1.0 are pre-registered)
    _cb = nc.alloc_sbuf_tensor(f"const-float32-{-M0}", [128, 1], F32)
    nc.gpsimd.memset(_cb.ap(), -M0)
    nc.const_aps.aps[(F32, -M0)] = _cb.ap()
    nc.all_engine_barrier()

    # x^T fp8, DoubleRow pairs: row kp*128+p, col i*B+b = x[b, (2kp+i)*128+p]
    xt = nc.declare_dram_parameter("xt", [256, 2 * B], F8, isOutput=False)
    # W^T fp8, DoubleRow pairs, superblock-major: per pair row-block and
    # superblock (c0, sw), cols [2*c0 : 2*c0+2*sw] hold [2, sw] row-major
    wt = nc.declare_dram_parameter("wt", [256, 2 * CS], F8, isOutput=False)
    # per-core partial Z sums, row p col bi = batch bi*128+p
    out = nc.declare_dram_parameter("out", [128, 4], F32, isOutput=True)

    with TileContext(nc, num_cores=NCORES) as tc:
        with (
            tc.tile_pool(name="xpool", bufs=1) as xpool,
            tc.tile_pool(name="wpool", bufs=3) as wpool,
            tc.tile_pool(name="epool", bufs=3) as epool,
            tc.tile_pool(name="zpool", bufs=1) as zpool,
            tc.tile_pool(name="psum", bufs=2, space="PSUM") as pp,
        ):
            # x^T fp8 pair tiles [K=128, sub=2, B]
            xts = []
            for kp in range(2):
                t = xpool.tile([128, 2, B], F8, tag=f"xt{kp}", name=f"xts{kp}")
                nc.sync.dma_start(
                    t[:],
                    xt[kp * 128 : (kp + 1) * 128, :].rearrange(
                        "p (s b) -> p s b", s=2
                    ),
                )
                xts.append(t)

            for rep in range(repeat):
                _body(nc, tc, rep, xpool, wpool, epool, zpool, pp, wt, out,
                      xts)

    if not nc.is_finalized():
        nc.finalize()
    return nc


def _body(nc, tc, rep, xpool, wpool, epool, zpool, pp, wt, out, xts):
    # per-batch-tile partial exp-sums, one col per superblock
    zbufs = [
        zpool.tile([128, 8], F32, tag=f"zb{bi}", name=f"zb{bi}_{rep}")
        for bi in range(4)
    ]

    c0 = 0
    for sbi, sw in enumerate(SBS):
        wts = []
        for kp in range(2):
            t = wpool.tile(
                [128, 2, SUPER], F8, tag=f"w{kp}", name=f"wts{kp}_{rep}"
            )
            nc.sync.dma_start(
                t[:, :, :sw],
                wt[
                    kp * 128 : (kp + 1) * 128, 2 * c0 : 2 * c0 + 2 * sw
                ].rearrange("p (s c) -> p s c", s=2),
            )
            wts.append(t)
        for bi in range(4):
            ps = pp.tile([128, SUPER], F32, tag="ps", name=f"ps_{rep}")
            for nb0 in range(0, sw, NBLK):
                nb = min(NBLK, sw - nb0)
                for kp in range(2):
                    nc.tensor.matmul(
                        ps[:, nb0 : nb0 + nb],
                        xts[kp][:, :, bi * 128 : (bi + 1) * 128],
                        wts[kp][:, :, nb0 : nb0 + nb],
                        start=(kp == 0),
                        stop=(kp == 1),
                        perf_mode=DR,
                    )
            ex = epool.tile([128, SUPER], BF16, tag="ex", name=f"ex_{rep}")
            nc.scalar.activation(
                ex[:, :sw], ps[:, :sw], FN.Exp, bias=-M0, scale=S / (QS * QS)
            )
            if USE_TTR:
                # fold the chunk in half and reduce in one DVE pass:
                # zbufs[col] = sum((ex[:, :h] + 0) + ex[:, h:2h])
                h = sw // 2
                sc = epool.tile([128, SUPER // 2], BF16, tag="tts",
                                name=f"tts_{rep}")
                nc.vector.scalar_tensor_tensor(
                    sc[:, :h], ex[:, :h], 0.0, ex[:, h : 2 * h],
                    ADD, ADD, accum_out=zbufs[bi][:, sbi : sbi + 1],
                )
            else:
                nc.vector.reduce_sum(
                    zbufs[bi][:, sbi : sbi + 1], ex[:, :sw],
                    axis=mybir.AxisListType.X,
                )
        c0 += sw

    # partial Z per core -> out[p, bi] = Z-partial of batch row bi*128+p
    zs_all = zpool.tile([128, 4], F32, tag="zsall", name=f"zsall_{rep}")
    for bi in range(4):
        nc.vector.reduce_sum(
            zs_all[:, bi : bi + 1], zbufs[bi][:, : len(SBS)],
            axis=mybir.AxisListType.X,
        )
    nc.sync.dma_start(out[:], zs_all[:])


def _dr_pack(aT):
    """[D, N] (D=512) -> [256, 2*N]: DoubleRow pair layout. Row kp*128+p,
    col i*N+n = aT[(2*kp+i)*128 + p, n]."""
    d, n = aT.shape
    chunks = aT.reshape(4, 128, n)
    pairs = [
        np.stack([chunks[2 * kp], chunks[2 * kp + 1]], axis=1).reshape(
            128, 2 * n
        )
        for kp in range(2)
    ]
    return np.concatenate(pairs, axis=0)


def _host_prep(input, label, weight):
    x = np.asarray(input, dtype=np.float32)
    lab = np.asarray(label).astype(np.int64).ravel()
    w = np.asarray(weight, dtype=np.float32)
    f8 = mybir.dt.np(F8)

    xn64 = x.astype(np.float64)
    xn64 /= np.maximum(
        np.sqrt(np.einsum("bd,bd->b", xn64, xn64))[:, None], 1e-12
    )
    xq = (xn64 * QS).astype(np.float32).astype(f8)  # [B, D] fp8
    xt = np.ascontiguousarray(_dr_pack(xq.astype(np.float32).T).astype(f8))

    wn_inv = 1.0 / np.maximum(
        np.sqrt(np.einsum("cd,cd->c", w, w, dtype=np.float64)), 1e-12
    )
    wn = w * wn_inv[:, None].astype(np.float32)  # [C, D] normalized rows, f32
    wq = (wn * QS).astype(f8)  # [C, D] fp8

    # label terms (tiny): phi from the exact f64 cosine, the Z-correction
    # from the fp8 cosine the device actually summed
    wl = wn[lab].astype(np.float64)  # [B, D]
    cosl = np.einsum("bd,bd->b", xn64, wl)
    cosl = np.clip(cosl, -1.0, 1.0)
    sine = np.sqrt(np.maximum(1.0 - cosl * cosl, 0.0))
    phi = cosl * COS_M - sine * SIN_M
    phi = np.where(cosl > TH, phi, cosl - MM)
    cosq = np.einsum(
        "bd,bd->b",
        xq.astype(np.float32),
        wq[lab].astype(np.float32),
        dtype=np.float64,
    ) / (QS * QS)
    post = {"phi": phi, "cosq": cosq}

    # class-sharded, transposed, DoubleRow-packed, superblock-major W
    shards = []
    for i in range(NCORES):
        lo, hi = i * CS, min((i + 1) * CS, C)
        sh = np.zeros((CS, D), dtype=f8)
        sh[: hi - lo] = wq[lo:hi]
        packed = _dr_pack(sh.astype(np.float32).T)  # [256, 2*CS], pair layout
        # rearrange cols to superblock-major [2, sw] blocks
        dst = np.empty_like(packed)
        q = 0
        c0 = 0
        for sw in SBS:
            blk = packed.reshape(256, 2, CS)[:, :, c0 : c0 + sw]
            dst[:, q : q + 2 * sw] = blk.reshape(256, 2 * sw)
            q += 2 * sw
            c0 += sw
        shards.append(np.ascontiguousarray(dst.astype(f8)))
    return xt, shards, post


def _finish(outs, post):
    """outs: per-core [128, 4] partial-Z blocks (device order p*4+bi).
    Returns the final loss (f64 host math)."""
    z = np.zeros((128, 4), dtype=np.float64)
    for o in outs:
        z += np.asarray(o, dtype=np.float64)
    zflat = z.reshape(512)  # index j -> batch (j%4)*128 + j//4
    perm = (np.arange(B) % 4) * 128 + np.arange(B) // 4
    Z = np.empty(B, dtype=np.float64)
    Z[perm] = zflat
    phi, cosq = post["phi"], post["cosq"]
    Zp = Z - np.exp(S * cosq - M0) + np.exp(S * phi - M0)
    nll = M0 + np.log(Zp) - S * phi
    return float(np.mean(nll))


def kernel(input, label, weight):
    global _GRAPH, LAST_RESULT
    xt, shards, post = _host_prep(input, label, weight)
    if _GRAPH is None:
        _GRAPH = _build_nc()
    in_maps = [{"xt": xt, "wt": shards[i]} for i in range(NCORES)]
    res = run_bass_kernel_spmd(_GRAPH, in_maps, list(range(NCORES)))
    LAST_RESULT = res
    loss = _finish([res.results[i]["out"] for i in range(NCORES)], post)
    return np.float32(loss).reshape(())

